# revision 21
# baseline (speedup 1.0000x reference)
"""Trainium2 Bass kernel for nn_MemoryNetwork (scatter_memory).

Reference computation (B=16384, I=2048, E=768, D=9, M=10, TAU=32):
    feat   = feature / ||feature||_2                       [B, I]
    mems_d = memory_tables[category[:9]]                   [D, M, E]  (first-9 quirk)
    t      = feat @ W_topic.T                              [B, E]
    att    = softmax(einsum('be,dme->bdm', t, mems_d)*TAU) [B, D, M]
    sep    = einsum('bdm,dme->bde', att, mems_d)           [B, D, E]
    dproj  = feat @ W_domain.T                             [B, E]
    out    = softmax(einsum('bde,be->bd', sep, dproj)*TAU) [B, 1, D]

Algebraic collapse (exact up to fp reassociation):
    K = [mems_d.reshape(90,E) @ W_topic; ... @ W_domain]   [180, I]
    G = feature @ K.T ;  logits scaled by r_b = TAU/||feature[b]||
    grouped softmaxes over (m in 10) then (d in 9).

Device strategy (per core, data-parallel over B):
  - feat is host-transposed to [i, b] layout and pre-rounded to the f32r
    grid (s1e8m11).  The main matmul runs in f32r at 1 cyc/row: for each
    b-tile (128 rows), 16 accumulating matmuls lhsT=featT block [i,b],
    rhs = K.T block [i, 256] where cols 0:180 hold K and cols 180:256 are
    an on-device DVE duplicate of cols 52:128 -- only present to reach
    the >=256 moving size required for the fast f32r path; never read.
  - K's f32r rounding error is corrected with a second accumulation pass
    into the same PSUM in bf16: dK = K - f32r(K) shipped as bf16, against
    a bf16 copy of feat produced on the (otherwise idle) ACT engine.
    This runs one b-tile behind the f32r pass so the conversion is off
    the critical path.  End-to-end error ~1.3e-2 (vs 2e-2 gate).
  - The last two b-tiles stream in kb-quarters (DMA + conversion +
    sweeps) to collapse the tail; the final softmax piece is a single
    b-tile with a fused exp+accumulate on ACT.
"""

import os
import sys

import numpy as np

for _p in ("/opt/trn_rl_repo", "/root/.axon_site/_ro/trn_rl_repo"):
    if os.path.isdir(_p) and _p not in sys.path:
        sys.path.insert(0, _p)

# A previously wedged NeuronCore (NRT_EXEC_UNIT_UNRECOVERABLE) recovers on
# the next open if cores are reset; harmless on a healthy device.
os.environ.setdefault("NEURON_RT_RESET_CORES", "1")

B, I, E = 16384, 2048, 768
D, M, TAU = 9, 10, 32.0
NCORES = 8
BLOC = B // NCORES          # 2048 rows per core
NT = BLOC // 128            # 16 b-tiles per core
KI = I // 128               # 16 contraction blocks
KR = 2 * D * M              # 180 = [A; C] rows
KPAD = 256                  # padded moving width for the f32r fast path
NTPC = 4                    # b-tiles per softmax chunk
NCHUNK = NT // NTPC         # 4
NWARM = 17                 # PE warm-up matmuls during the DMA lead-in
NQT = 2                     # last NQT tiles stream in kb-quarters

_NC_CACHE = {}


def _build_nc():
    import concourse.bass as bass
    import concourse.mybir as mybir
    import concourse.tile as tile

    fp32 = mybir.dt.float32
    f32r = mybir.dt.float32r
    bf16 = mybir.dt.bfloat16
    Alu = mybir.AluOpType
    Act = mybir.ActivationFunctionType

    nc = bass.Bass()
    # [p, j, kb, b] = f32r(feature[core*BLOC + j*128 + b, kb*128 + p])
    feat = nc.dram_tensor("feat", [128, NT * I], f32r, kind="ExternalInput")
    # [r (16 cols) | warm-up junk (128 cols)]
    cst = nc.dram_tensor("cst", [128, NT + 128], fp32, kind="ExternalInput")
    # [p, kb, n] = f32r(K[n, kb*128+p]), n < 180
    ktr = nc.dram_tensor("ktr", [128, KI * KR], f32r, kind="ExternalInput")
    # [p, kb, m] = bf16(K[m, kb*128+p] - f32r(K[m, kb*128+p]))
    dkb = nc.dram_tensor("dkb", [128, KI * KR], bf16, kind="ExternalInput")
    # [p, j, d]; host permutes back to [BLOC, D]
    out = nc.dram_tensor("out", [128, NT * D], fp32, kind="ExternalOutput")

    with tile.TileContext(nc) as tc:
        with (
            tc.tile_pool(name="const", bufs=1) as cpool,
            tc.tile_pool(name="nat", bufs=16) as natp,
            tc.tile_pool(name="fbp", bufs=4) as fbp,
            tc.tile_pool(name="scp", bufs=4) as scp,
            tc.tile_pool(name="stp", bufs=6) as stp,
            tc.tile_pool(name="pG", bufs=1, space="PSUM") as pG,
            tc.tile_pool(name="pJ", bufs=1, space="PSUM") as pJ,
        ):
            ot_a = cpool.tile([128, NT - 2, D], fp32)
            ot_b = cpool.tile([128, 2, D], fp32)

            cst_sb = cpool.tile([128, NT + 128], fp32)
            kt = cpool.tile([128, KI, KPAD], f32r)
            dk = cpool.tile([128, KI, KR], bf16)
            nc.sync.dma_start(cst_sb, cst[:, :])
            nc.sync.dma_start(kt[:, :, :KR],
                              ktr[:, :].rearrange("p (k n) -> p k n", n=KR))
            r_sb = cst_sb[:, :NT]
            warm = cst_sb[:, NT:]

            # ---- PE warm-up: absorb the cst DMA wait, then keep the PE
            # busy through the DMA lead-in so the p-state ramp finishes
            # before real work starts.
            pw = pJ.tile([128, 128], fp32, tag="pw", name="pw")
            pjk = pJ.tile([128, 2], fp32, tag="pjk", name="pjk")
            for _w in range(NWARM):
                nc.tensor.matmul(pw, warm, warm, start=True, stop=True)
            # DVE absorber for the cst DMA wait (evictions read r_sb) and
            # ACT absorber for the same (fb-junk writes read r_sb)
            rjk = cpool.tile([128, 2], fp32)
            nc.vector.tensor_copy(rjk[:, 0:1], r_sb[:, 0:1])
            nc.scalar.activation(rjk[:, 1:2], r_sb[:, 0:1],
                                 Act.Identity)
            # fill cols 180:256 of kt with a duplicate of cols 52:128 so the
            # f32r moving operand is one contiguous 256-wide AP
            nc.vector.tensor_copy(kt[:, :, KR:], kt[:, :, 52:52 + KPAD - KR])
            # PE absorbers: ktr DMA wait, then the DVE-dup wait, so tile
            # 0's first real matmul carries no extra wait
            nc.tensor.matmul(pjk[:1, :], kt[:, 0, 0:1], kt[:, 0, 0:2],
                             start=True, stop=True)
            nc.tensor.matmul(pjk[:1, :], kt[:, 0, KR:KR + 1],
                             kt[:, 0, KR:KR + 2], start=True, stop=True)

            gp_t = [pG.tile([128, KPAD], fp32, tag=f"gp{k}", name=f"gp{k}")
                    for k in range(3)]

            ft_t = []
            fb_t = []
            sc_list = []

            def load_tile(j):
                ft = natp.tile([128, KI, 128], f32r, tag="ft", name="ft")
                src = feat[:, j * I:(j + 1) * I].rearrange(
                    "p (k b) -> p k b", b=128)
                if j >= NT - NQT:
                    for q in range(4):
                        nc.sync.dma_start(ft[:, 4 * q:4 * (q + 1), :],
                                          src[:, 4 * q:4 * (q + 1), :])
                else:
                    nc.sync.dma_start(ft, src)
                ft_t.append(ft)

            def convert_tile(j):
                # bf16 copy of the f32r feat tile for the correction pass.
                # Mid-kernel tiles convert whole on ACT; the two tail tiles
                # convert in quarters on separate engines (14 -> DVE,
                # 15 -> ACT) so their FIFOs don't serialize the tail.
                fb = fbp.tile([128, KI, 128], bf16, tag="fb", name="fb")
                if j >= 4:
                    # fb buffer reuse: a junk write (reading the long-ready
                    # r_sb) absorbs the WAR wait on the old corr sweep, so
                    # the conversion itself carries only its feat-DMA wait
                    if j == NT - 2:
                        nc.vector.tensor_copy(fb[:, 0, 0:1], rjk[:, 0:1])
                    else:
                        nc.scalar.activation(fb[:, 0, 0:1], rjk[:, 1:2],
                                             Act.Identity)
                if j >= NT - NQT:
                    for q in range(4):
                        src = ft_t[j][:, 4 * q:4 * (q + 1), :].bitcast(fp32)
                        dst = fb[:, 4 * q:4 * (q + 1), :]
                        if j == NT - 2:
                            nc.vector.tensor_copy(dst, src)
                        else:
                            nc.scalar.activation(dst, src, Act.Identity)
                else:
                    nc.scalar.activation(fb, ft_t[j].bitcast(fp32),
                                         Act.Identity)
                fb_t.append(fb)

            def main_quarter(j, q, nquart):
                gp = gp_t[j % 3]
                kb0 = q * (KI // nquart)
                # absorb the feat-(quarter-)DMA wait
                nc.tensor.matmul(pjk[:1, :], ft_t[j][:, kb0, 0:1],
                                 ft_t[j][:, kb0, 0:2],
                                 start=True, stop=True)
                for kb in range(kb0, kb0 + KI // nquart):
                    nc.tensor.matmul(gp, ft_t[j][:, kb, :], kt[:, kb, :],
                                     start=(kb == 0), stop=False,
                                     skip_group_check=True)

            def corr_quarter(j, q, nquart):
                gp = gp_t[j % 3]
                kb0 = q * (KI // nquart)
                for kb in range(kb0, kb0 + KI // nquart):
                    nc.tensor.matmul(gp[:, :KR], fb_t[j][:, kb, :],
                                     dk[:, kb, :], start=False,
                                     stop=(kb == KI - 1),
                                     skip_group_check=True)

            def main_sweep(j):
                nquart = 4 if j >= NT - NQT else 1
                for q in range(nquart):
                    main_quarter(j, q, nquart)

            def corr_sweep(j):
                nquart = 4 if j >= NT - NQT else 1
                for q in range(nquart):
                    corr_quarter(j, q, nquart)

            def evict(j):
                ci, jj = divmod(j, NTPC)
                nc.vector.tensor_scalar_mul(
                    sc_list[ci][:, jj, :], gp_t[j % 3][:, :KR],
                    r_sb[:, j:j + 1])

            def softmax_chunk(ci, j0=0, j1=NTPC):
                sc = sc_list[ci]
                nj = j1 - j0
                S = sc[:, j0:j1, 0:90].rearrange("p c (d m) -> p c d m", m=M)
                C_ = sc[:, j0:j1, 90:180].rearrange("p c (d m) -> p c d m",
                                                    m=M)
                sh4 = (128, nj, D, M)
                mx = stp.tile([128, nj, D], fp32, tag="mx", name="mx")
                nc.vector.tensor_reduce(mx, S, axis=mybir.AxisListType.X,
                                        op=Alu.max)
                nc.vector.tensor_tensor(
                    S, S, mx[:, :, :, None].to_broadcast(sh4), Alu.subtract)
                ex = stp.tile([128, nj, D, M], fp32, tag="ex", name="ex")
                nc.scalar.activation(ex, S, Act.Exp)
                den = stp.tile([128, nj, D], fp32, tag="den", name="den")
                nc.vector.tensor_reduce(den, ex, axis=mybir.AxisListType.X,
                                        op=Alu.add)
                ec = stp.tile([128, nj, D, M], fp32, tag="ec", name="ec")
                nc.vector.tensor_tensor(ec, ex, C_, Alu.mult)
                num = stp.tile([128, nj, D], fp32, tag="num", name="num")
                nc.vector.tensor_reduce(num, ec, axis=mybir.AxisListType.X,
                                        op=Alu.add)
                rden = stp.tile([128, nj, D], fp32, tag="rden", name="rden")
                nc.vector.reciprocal(rden, den)
                L = stp.tile([128, nj, D], fp32, tag="L", name="L")
                nc.vector.tensor_tensor(L, num, rden, Alu.mult)
                sh3 = (128, nj, D)
                mx2 = stp.tile([128, nj], fp32, tag="mx2", name="mx2")
                nc.vector.tensor_reduce(mx2, L, axis=mybir.AxisListType.X,
                                        op=Alu.max)
                nc.vector.tensor_tensor(
                    L, L, mx2[:, :, None].to_broadcast(sh3), Alu.subtract)
                e2 = stp.tile([128, nj, D], fp32, tag="e2", name="e2")
                den2 = stp.tile([128, nj], fp32, tag="den2", name="den2")
                if nj == 1:
                    # fused exp + per-partition accumulate on ACT
                    nc.scalar.activation(e2, L, Act.Exp,
                                         accum_out=den2[:, 0:1])
                else:
                    nc.scalar.activation(e2, L, Act.Exp)
                    nc.vector.tensor_reduce(den2, e2,
                                            axis=mybir.AxisListType.X,
                                            op=Alu.add)
                rden2 = stp.tile([128, nj], fp32, tag="rden2", name="rden2")
                nc.vector.reciprocal(rden2, den2)
                r0 = ci * NTPC + j0
                if r0 >= NT - 2:
                    dst = ot_b[:, r0 - (NT - 2):r0 - (NT - 2) + nj, :]
                else:
                    dst = ot_a[:, r0:r0 + nj, :]
                nc.vector.tensor_tensor(
                    dst, e2, rden2[:, :, None].to_broadcast(sh3), Alu.mult)

            for ci in range(NCHUNK):
                sc_list.append(scp.tile([128, NTPC, KR], fp32, tag="sc",
                                        name="sc"))

            # DMA order: cst | ktr | feat0 | dkb | feat1..15
            load_tile(0)
            nc.sync.dma_start(dk, dkb[:, :].rearrange("p (k m) -> p k m",
                                                      m=KR))
            for j in range(1, NT):
                load_tile(j)

            # absorber for the dkb DMA wait (before the first corr sweep)
            nc.tensor.matmul(pjk[:1, :], dk[:, 0, 0:1], dk[:, 0, 0:2],
                             start=True, stop=True)

            # software-pipelined main/correction sweeps: f32r sweep j runs
            # while tile j-1's bf16 correction + eviction follow behind.
            outv = out[:, :].rearrange("p (t d) -> p t d", d=D)
            for j in range(NT - 1):
                if j >= 1:
                    jm = j - 1
                    corr_sweep(jm)
                    evict(jm)
                    if jm == NTPC - 1:
                        softmax_chunk(0)
                    elif jm == 2 * NTPC - 1:
                        softmax_chunk(1)
                    elif jm == 3 * NTPC - 1:
                        softmax_chunk(2)
                    elif jm == NT - 4:
                        softmax_chunk(3, 0, 1)
                    elif jm == NT - 3:
                        softmax_chunk(3, 1, 2)
                        nc.sync.dma_start(outv[:, :NT - 2, :], ot_a)
                convert_tile(j)
                main_sweep(j)
            # ---- tail choreography (j = NT-1): interleave tile-14's
            # correction and tile-15's arrival-gated main/correction
            # quarters by expected data readiness.
            jl = NT - 1
            convert_tile(jl)
            corr_quarter(jl - 1, 0, 4)
            corr_quarter(jl - 1, 1, 4)
            main_quarter(jl, 0, 4)
            corr_quarter(jl - 1, 2, 4)
            corr_quarter(jl - 1, 3, 4)
            evict(jl - 1)
            softmax_chunk(3, 2, 3)
            main_quarter(jl, 1, 4)
            main_quarter(jl, 2, 4)
            main_quarter(jl, 3, 4)
            corr_quarter(jl, 0, 4)
            corr_quarter(jl, 1, 4)
            corr_quarter(jl, 2, 4)
            corr_quarter(jl, 3, 4)
            evict(jl)
            softmax_chunk(3, 3, NTPC)
            nc.sync.dma_start(outv[:, NT - 2:, :], ot_b)

    # Post-pass: walrus's codegen allows at most ONE embedded sync wait per
    # instruction (S3_LW / S3D3 / DMA structs alike).  For any instruction
    # with more, hoist all but the last wait into same-engine InstDrain ops
    # inserted just before it -- engine program order preserves semantics.
    import concourse.mybir as mybir
    for fn in nc.m.functions:
        for blk in fn.blocks:
            lst = blk.instructions
            k = 0
            while k < len(lst):
                ins = lst[k]
                si = ins.sync_info
                if (si is not None and si.on_wait and len(si.on_wait) > 1):
                    w = list(si.on_wait)
                    ups = list(si.on_update or [])
                    ins.sync_info = mybir.SyncInfo(on_wait=[w[-1]],
                                                   on_update=ups)
                    for j, wx in enumerate(w[:-1]):
                        lst.insert(k + j, mybir.InstDrain(
                            name=f"{ins.name}-sw{j}", engine=ins.engine,
                            sync_info=mybir.SyncInfo(on_wait=[wx],
                                                     on_update=[])))
                    k += len(w) - 1
                k += 1

    return nc


def _get_nc():
    if "nc" not in _NC_CACHE:
        _NC_CACHE["nc"] = _build_nc()
    return _NC_CACHE["nc"]


def _round_f32r(a):
    """Round-to-nearest-even onto the f32r (s1e8m11) grid."""
    a = np.ascontiguousarray(np.asarray(a, np.float32))
    bits = a.view(np.uint32).astype(np.uint64)
    lsb = (bits >> 12) & 1
    r = (bits + 0x7FF + lsb) & np.uint64(0xFFFFF000)
    return r.astype(np.uint32).view(np.float32)


def _host_prep(feature, W_topic, W_domain, memory_tables, category):
    import ml_dtypes

    feature = np.ascontiguousarray(np.asarray(feature, dtype=np.float32))
    cat = np.asarray(category).astype(np.int64)
    mems = np.asarray(memory_tables, dtype=np.float32)[cat[:D]]
    mf = mems.reshape(D * M, E).astype(np.float64)
    K = np.concatenate([mf @ np.asarray(W_topic, dtype=np.float64),
                        mf @ np.asarray(W_domain, dtype=np.float64)],
                       axis=0)                                   # [180, I]
    Kr = _round_f32r(K.astype(np.float32))                       # f32r grid
    dK = (K - Kr.astype(np.float64)).astype(np.float32)
    dKb = dK.astype(ml_dtypes.bfloat16)

    # ktr[p, kb, n] = Kr[n, kb*128+p]
    kr_t = np.ascontiguousarray(
        Kr.T.reshape(KI, 128, KR).transpose(1, 0, 2)).reshape(128, KI * KR)
    dk_t = np.ascontiguousarray(
        dKb.T.reshape(KI, 128, KR).transpose(1, 0, 2)).reshape(128, KI * KR)

    norm = np.sqrt(np.einsum("bi,bi->b", feature, feature,
                             dtype=np.float64))
    r = (TAU / np.maximum(norm, 1e-12)).astype(np.float32)       # [B]
    rsc = r.reshape(NCORES, NT, 128).transpose(0, 2, 1)          # [NC,128,NT]
    warm = np.broadcast_to(np.eye(128, dtype=np.float32)[None],
                           (NCORES, 128, 128))
    cst = np.ascontiguousarray(np.concatenate([rsc, warm], axis=2))

    # featT packed per core: [p, j, kb, b]
    fr = _round_f32r(feature)
    ft = fr.reshape(NCORES, NT, 128, KI, 128).transpose(0, 4, 1, 3, 2)
    ft = np.ascontiguousarray(ft).reshape(NCORES, 128, NT * I)
    return ft, cst, kr_t, dk_t


def _run(ft, cst, kr_t, dk_t, trace=False):
    from concourse.bass_utils import run_bass_kernel_spmd

    nc = _get_nc()
    in_maps = [
        {"feat": ft[c], "cst": cst[c], "ktr": kr_t, "dkb": dk_t}
        for c in range(NCORES)
    ]
    res = run_bass_kernel_spmd(nc, in_maps, core_ids=list(range(NCORES)),
                               trace=trace)
    # out[p, j, d] -> [BLOC, D] per core
    outs = []
    for c in range(NCORES):
        o = res.results[c]["out"].reshape(128, NT, D)
        outs.append(o.transpose(1, 0, 2).reshape(BLOC, D))
    out = np.concatenate(outs, axis=0)
    return out.reshape(B, 1, D), res


def kernel(feature=None, W_topic=None, W_domain=None, memory_tables=None,
           category=None, **_unused):
    prep = _host_prep(feature, W_topic, W_domain, memory_tables, category)
    out, _ = _run(*prep, trace=False)
    return out


# revision 28
# speedup vs baseline: 1.0314x; 1.0314x over previous
"""Trainium2 Bass kernel for nn_MemoryNetwork (scatter_memory).

Reference computation (B=16384, I=2048, E=768, D=9, M=10, TAU=32):
    feat   = feature / ||feature||_2                       [B, I]
    mems_d = memory_tables[category[:9]]                   [D, M, E]  (first-9 quirk)
    t      = feat @ W_topic.T                              [B, E]
    att    = softmax(einsum('be,dme->bdm', t, mems_d)*TAU) [B, D, M]
    sep    = einsum('bdm,dme->bde', att, mems_d)           [B, D, E]
    dproj  = feat @ W_domain.T                             [B, E]
    out    = softmax(einsum('bde,be->bd', sep, dproj)*TAU) [B, 1, D]

Algebraic collapse (exact up to fp reassociation):
    K = [mems_d.reshape(90,E) @ W_topic; ... @ W_domain]   [180, I]
    G = feature @ K.T ;  logits scaled by r_b = TAU/||feature[b]||
    grouped softmaxes over (m in 10) then (d in 9).

Device strategy (per core, data-parallel over B):
  - feat is host-transposed to [i, b] layout and pre-rounded to the f32r
    grid (s1e8m11).  The main matmul runs in f32r at 1 cyc/row: for each
    b-tile (128 rows), 16 accumulating matmuls lhsT=featT block [i,b],
    rhs = K.T block [i, 256] where cols 0:180 hold K and cols 180:256 are
    an on-device DVE duplicate of cols 52:128 -- only present to reach
    the >=256 moving size required for the fast f32r path; never read.
  - K's f32r rounding error is corrected with a second accumulation pass
    into the same PSUM in bf16: dK = K - f32r(K) shipped as bf16, against
    a bf16 copy of feat produced on the (otherwise idle) ACT engine.
    This runs one b-tile behind the f32r pass so the conversion is off
    the critical path.  End-to-end error ~1.3e-2 (vs 2e-2 gate).
  - The last two b-tiles stream in kb-quarters (DMA + conversion +
    sweeps) to collapse the tail; the final softmax piece is a single
    b-tile with a fused exp+accumulate on ACT.
"""

import os
import sys

import numpy as np

for _p in ("/opt/trn_rl_repo", "/root/.axon_site/_ro/trn_rl_repo"):
    if os.path.isdir(_p) and _p not in sys.path:
        sys.path.insert(0, _p)

# A previously wedged NeuronCore (NRT_EXEC_UNIT_UNRECOVERABLE) recovers on
# the next open if cores are reset; harmless on a healthy device.
os.environ.setdefault("NEURON_RT_RESET_CORES", "1")

B, I, E = 16384, 2048, 768
D, M, TAU = 9, 10, 32.0
NCORES = 8
BLOC = B // NCORES          # 2048 rows per core
NT = BLOC // 128            # 16 b-tiles per core
KI = I // 128               # 16 contraction blocks
KR = 2 * D * M              # 180 = [A; C] rows
KPAD = 256                  # padded moving width for the f32r fast path
KRP = 192                   # corr width padded for DoubleRow step%16
NTPC = 4                    # b-tiles per softmax chunk
NCHUNK = NT // NTPC         # 4
NWARM = 17                 # PE warm-up matmuls during the DMA lead-in
NQT = 2                     # last NQT tiles stream in kb-quarters

_NC_CACHE = {}


def _build_nc():
    import concourse.bass as bass
    import concourse.mybir as mybir
    import concourse.tile as tile

    fp32 = mybir.dt.float32
    fp8 = mybir.dt.float8e4
    f32r = mybir.dt.float32r
    bf16 = mybir.dt.bfloat16
    Alu = mybir.AluOpType
    Act = mybir.ActivationFunctionType

    nc = bass.Bass()
    # [p, j, kb, b] = f32r(feature[core*BLOC + j*128 + b, kb*128 + p])
    feat = nc.dram_tensor("feat", [128, NT * I], f32r, kind="ExternalInput")
    # [r (16 cols) | warm-up junk (128 cols)]
    cst = nc.dram_tensor("cst", [128, NT + 128], fp32, kind="ExternalInput")
    # [p, kb, n] = f32r(K[n, kb*128+p]), n < 180
    ktr = nc.dram_tensor("ktr", [128, KI * KR], f32r, kind="ExternalInput")
    # [p, pair, ko, m] = fp8e4((K - f32r(K))[m, (2*pair+ko)*128+p] * 4096),
    # m padded to KRP for the DoubleRow step%16 rule
    dk8 = nc.dram_tensor("dk8", [128, (KI // 2) * 2 * KRP], fp8,
                         kind="ExternalInput")
    # pre-converted fp8 copy of the LAST feat tile (kills the tail
    # conversion chain): [p, kb, b]
    fb8d = nc.dram_tensor("fb8d", [128, I], fp8, kind="ExternalInput")
    # [p, j, d]; host permutes back to [BLOC, D]
    out = nc.dram_tensor("out", [128, NT * D], fp32, kind="ExternalOutput")

    with tile.TileContext(nc) as tc:
        with (
            tc.tile_pool(name="const", bufs=1) as cpool,
            tc.tile_pool(name="nat", bufs=16) as natp,
            tc.tile_pool(name="fbp", bufs=4) as fbp,
            tc.tile_pool(name="fb15p", bufs=1) as fb15p,
            tc.tile_pool(name="scp", bufs=4) as scp,
            tc.tile_pool(name="stp", bufs=6) as stp,
            tc.tile_pool(name="pG", bufs=1, space="PSUM") as pG,
            tc.tile_pool(name="pJ", bufs=1, space="PSUM") as pJ,
        ):
            ot_a = cpool.tile([128, NT - 2, D], fp32)
            ot_b = cpool.tile([128, 2, D], fp32)

            cst_sb = cpool.tile([128, NT + 128], fp32)
            kt = cpool.tile([128, KI, KPAD], f32r)
            dk = cpool.tile([128, KI // 2, 2, KRP], fp8)
            nc.sync.dma_start(cst_sb, cst[:, :])
            nc.sync.dma_start(kt[:, :, :KR],
                              ktr[:, :].rearrange("p (k n) -> p k n", n=KR))
            r_sb = cst_sb[:, :NT]
            warm = cst_sb[:, NT:]

            # ---- PE warm-up: absorb the cst DMA wait, then keep the PE
            # busy through the DMA lead-in so the p-state ramp finishes
            # before real work starts.
            pw = pJ.tile([128, 128], fp32, tag="pw", name="pw")
            pjk = pJ.tile([128, 2], fp32, tag="pjk", name="pjk")
            for _w in range(NWARM):
                nc.tensor.matmul(pw, warm, warm, start=True, stop=True)
            # DVE absorber for the cst DMA wait (evictions read r_sb) and
            # ACT absorber for the same (fb-junk writes read r_sb)
            rjk = cpool.tile([128, 2], fp32)
            nc.vector.tensor_copy(rjk[:, 0:1], r_sb[:, 0:1])
            nc.scalar.activation(rjk[:, 1:2], r_sb[:, 0:1],
                                 Act.Identity)
            # fill cols 180:256 of kt with a duplicate of cols 52:128 so the
            # f32r moving operand is one contiguous 256-wide AP
            nc.vector.tensor_copy(kt[:, :, KR:], kt[:, :, 52:52 + KPAD - KR])
            # PE absorbers: ktr DMA wait, then the DVE-dup wait, so tile
            # 0's first real matmul carries no extra wait
            nc.tensor.matmul(pjk[:1, :], kt[:, 0, 0:1], kt[:, 0, 0:2],
                             start=True, stop=True)
            nc.tensor.matmul(pjk[:1, :], kt[:, 0, KR:KR + 1],
                             kt[:, 0, KR:KR + 2], start=True, stop=True)

            gp_t = [pG.tile([128, KPAD], fp32, tag=f"gp{k}", name=f"gp{k}")
                    for k in range(3)]

            ft_t = []
            fb_t = []
            sc_list = []

            def load_tile(j):
                ft = natp.tile([128, KI, 128], f32r, tag="ft", name="ft")
                src = feat[:, j * I:(j + 1) * I].rearrange(
                    "p (k b) -> p k b", b=128)
                if j >= NT - NQT:
                    for q in range(4):
                        nc.sync.dma_start(ft[:, 4 * q:4 * (q + 1), :],
                                          src[:, 4 * q:4 * (q + 1), :])
                else:
                    nc.sync.dma_start(ft, src)
                ft_t.append(ft)

            def convert_tile(j):
                # bf16 copy of the f32r feat tile for the correction pass.
                # Mid-kernel tiles convert whole on ACT; the two tail tiles
                # convert in quarters on separate engines (14 -> DVE,
                # 15 -> ACT) so their FIFOs don't serialize the tail.
                if j == NT - 1:
                    fb_t.append(fb15)
                    return
                fb = fbp.tile([128, KI, 128], fp8, tag="fb", name="fb")
                if j >= 4:
                    # fb buffer reuse: a junk write (reading the long-ready
                    # r_sb) absorbs the WAR wait on the old corr sweep, so
                    # the conversion itself carries only its feat-DMA wait
                    nc.scalar.activation(fb[:, 0, 0:1], rjk[:, 1:2],
                                         Act.Identity)
                if j >= NT - NQT:
                    for q in range(4):
                        src = ft_t[j][:, 4 * q:4 * (q + 1), :].bitcast(fp32)
                        dst = fb[:, 4 * q:4 * (q + 1), :]
                        nc.scalar.activation(dst, src, Act.Identity)
                else:
                    nc.scalar.activation(fb, ft_t[j].bitcast(fp32),
                                         Act.Identity)
                fb_t.append(fb)

            def main_quarter(j, q, nquart):
                gp = gp_t[j % 3]
                kb0 = q * (KI // nquart)
                # absorb the feat-(quarter-)DMA wait
                nc.tensor.matmul(pjk[:1, :], ft_t[j][:, kb0, 0:1],
                                 ft_t[j][:, kb0, 0:2],
                                 start=True, stop=True)
                for kb in range(kb0, kb0 + KI // nquart):
                    nc.tensor.matmul(gp, ft_t[j][:, kb, :], kt[:, kb, :],
                                     start=(kb == 0), stop=False,
                                     skip_group_check=True)

            def corr_quarter(j, q, nquart):
                gp = gp_t[j % 3]
                npair = KI // 2
                p0 = q * (npair // nquart)
                for pr in range(p0, p0 + npair // nquart):
                    nc.tensor.matmul(gp[:, :KRP],
                                     fb_t[j][:, 2 * pr:2 * pr + 2, :],
                                     dk[:, pr, :, :], start=False,
                                     stop=(pr == npair - 1),
                                     skip_group_check=True,
                                     perf_mode=mybir.MatmulPerfMode.DoubleRow)

            def main_sweep(j):
                nquart = 4 if j >= NT - NQT else 1
                for q in range(nquart):
                    main_quarter(j, q, nquart)

            def corr_sweep(j):
                nquart = 4 if j >= NT - NQT else 1
                for q in range(nquart):
                    corr_quarter(j, q, nquart)

            def evict(j):
                # gp holds 4096*(f.Kr' + f8.dKs) -- one r/4096 row scale
                ci, jj = divmod(j, NTPC)
                nc.vector.tensor_scalar_mul(
                    sc_list[ci][:, jj, :], gp_t[j % 3][:, :KR],
                    r_sb[:, j:j + 1])

            def softmax_chunk(ci, j0=0, j1=NTPC):
                sc = sc_list[ci]
                nj = j1 - j0
                S = sc[:, j0:j1, 0:90].rearrange("p c (d m) -> p c d m", m=M)
                C_ = sc[:, j0:j1, 90:180].rearrange("p c (d m) -> p c d m",
                                                    m=M)
                sh4 = (128, nj, D, M)
                mx = stp.tile([128, nj, D], fp32, tag="mx", name="mx")
                nc.vector.tensor_reduce(mx, S, axis=mybir.AxisListType.X,
                                        op=Alu.max)
                nc.vector.tensor_tensor(
                    S, S, mx[:, :, :, None].to_broadcast(sh4), Alu.subtract)
                ex = stp.tile([128, nj, D, M], fp32, tag="ex", name="ex")
                nc.scalar.activation(ex, S, Act.Exp)
                den = stp.tile([128, nj, D], fp32, tag="den", name="den")
                nc.vector.tensor_reduce(den, ex, axis=mybir.AxisListType.X,
                                        op=Alu.add)
                ec = stp.tile([128, nj, D, M], fp32, tag="ec", name="ec")
                nc.vector.tensor_tensor(ec, ex, C_, Alu.mult)
                num = stp.tile([128, nj, D], fp32, tag="num", name="num")
                nc.vector.tensor_reduce(num, ec, axis=mybir.AxisListType.X,
                                        op=Alu.add)
                rden = stp.tile([128, nj, D], fp32, tag="rden", name="rden")
                nc.vector.reciprocal(rden, den)
                L = stp.tile([128, nj, D], fp32, tag="L", name="L")
                nc.vector.tensor_tensor(L, num, rden, Alu.mult)
                sh3 = (128, nj, D)
                mx2 = stp.tile([128, nj], fp32, tag="mx2", name="mx2")
                nc.vector.tensor_reduce(mx2, L, axis=mybir.AxisListType.X,
                                        op=Alu.max)
                nc.vector.tensor_tensor(
                    L, L, mx2[:, :, None].to_broadcast(sh3), Alu.subtract)
                e2 = stp.tile([128, nj, D], fp32, tag="e2", name="e2")
                den2 = stp.tile([128, nj], fp32, tag="den2", name="den2")
                if nj == 1:
                    # fused exp + per-partition accumulate on ACT
                    nc.scalar.activation(e2, L, Act.Exp,
                                         accum_out=den2[:, 0:1])
                else:
                    nc.scalar.activation(e2, L, Act.Exp)
                    nc.vector.tensor_reduce(den2, e2,
                                            axis=mybir.AxisListType.X,
                                            op=Alu.add)
                rden2 = stp.tile([128, nj], fp32, tag="rden2", name="rden2")
                nc.vector.reciprocal(rden2, den2)
                r0 = ci * NTPC + j0
                if r0 >= NT - 2:
                    dst = ot_b[:, r0 - (NT - 2):r0 - (NT - 2) + nj, :]
                else:
                    dst = ot_a[:, r0:r0 + nj, :]
                nc.vector.tensor_tensor(
                    dst, e2, rden2[:, :, None].to_broadcast(sh3), Alu.mult)

            for ci in range(NCHUNK):
                sc_list.append(scp.tile([128, NTPC, KR], fp32, tag="sc",
                                        name="sc"))

            # DMA order: cst | ktr | feat0 | dk8 | feat1..13 | fb8(15) |
            # feat14 quarters | feat15 quarters
            load_tile(0)
            nc.sync.dma_start(
                dk, dk8[:, :].rearrange("p (k o m) -> p k o m", o=2, m=KRP))
            for j in range(1, NT - 2):
                load_tile(j)
            fb15 = fb15p.tile([128, KI, 128], fp8)
            nc.sync.dma_start(
                fb15, fb8d[:, :].rearrange("p (k b) -> p k b", b=128))
            load_tile(NT - 2)
            load_tile(NT - 1)

            # absorber for the dkb DMA wait (before the first corr sweep)
            nc.tensor.matmul(pjk[:1, :], dk[:, 0, 0, 0:1], dk[:, 0, 0, 0:2],
                             start=True, stop=True)

            # software-pipelined main/correction sweeps: f32r sweep j runs
            # while tile j-1's bf16 correction + eviction follow behind.
            outv = out[:, :].rearrange("p (t d) -> p t d", d=D)
            for j in range(NT - 1):
                if j >= 1:
                    jm = j - 1
                    corr_sweep(jm)
                    evict(jm)
                    if jm == NTPC - 1:
                        softmax_chunk(0)
                    elif jm == 2 * NTPC - 1:
                        softmax_chunk(1)
                    elif jm == 3 * NTPC - 1:
                        softmax_chunk(2)
                    elif jm == NT - 4:
                        softmax_chunk(3, 0, 1)
                    elif jm == NT - 3:
                        softmax_chunk(3, 1, 2)
                        nc.sync.dma_start(outv[:, :NT - 2, :], ot_a)
                convert_tile(j)
                main_sweep(j)
            # ---- tail choreography (j = NT-1): interleave tile-14's
            # correction and tile-15's arrival-gated main/correction
            # quarters by expected data readiness.
            jl = NT - 1
            convert_tile(jl)
            corr_quarter(jl - 1, 0, 4)
            corr_quarter(jl - 1, 1, 4)
            main_quarter(jl, 0, 4)
            corr_quarter(jl - 1, 2, 4)
            corr_quarter(jl - 1, 3, 4)
            evict(jl - 1)
            softmax_chunk(3, 2, 3)
            main_quarter(jl, 1, 4)
            main_quarter(jl, 2, 4)
            main_quarter(jl, 3, 4)
            corr_quarter(jl, 0, 4)
            corr_quarter(jl, 1, 4)
            corr_quarter(jl, 2, 4)
            corr_quarter(jl, 3, 4)
            evict(jl)
            softmax_chunk(3, 3, NTPC)
            nc.sync.dma_start(outv[:, NT - 2:, :], ot_b)

    # Post-pass: walrus's codegen allows at most ONE embedded sync wait per
    # instruction (S3_LW / S3D3 / DMA structs alike).  For any instruction
    # with more, hoist all but the last wait into same-engine InstDrain ops
    # inserted just before it -- engine program order preserves semantics.
    import concourse.mybir as mybir
    for fn in nc.m.functions:
        for blk in fn.blocks:
            lst = blk.instructions
            k = 0
            while k < len(lst):
                ins = lst[k]
                si = ins.sync_info
                if (si is not None and si.on_wait and len(si.on_wait) > 1):
                    w = list(si.on_wait)
                    ups = list(si.on_update or [])
                    ins.sync_info = mybir.SyncInfo(on_wait=[w[-1]],
                                                   on_update=ups)
                    for j, wx in enumerate(w[:-1]):
                        lst.insert(k + j, mybir.InstDrain(
                            name=f"{ins.name}-sw{j}", engine=ins.engine,
                            sync_info=mybir.SyncInfo(on_wait=[wx],
                                                     on_update=[])))
                    k += len(w) - 1
                k += 1

    return nc


def _get_nc():
    if "nc" not in _NC_CACHE:
        _NC_CACHE["nc"] = _build_nc()
    return _NC_CACHE["nc"]


def _round_f32r(a):
    """Round-to-nearest-even onto the f32r (s1e8m11) grid."""
    a = np.ascontiguousarray(np.asarray(a, np.float32))
    bits = a.view(np.uint32).astype(np.uint64)
    lsb = (bits >> 12) & 1
    r = (bits + 0x7FF + lsb) & np.uint64(0xFFFFF000)
    return r.astype(np.uint32).view(np.float32)


def _host_prep(feature, W_topic, W_domain, memory_tables, category):
    import ml_dtypes

    feature = np.ascontiguousarray(np.asarray(feature, dtype=np.float32))
    cat = np.asarray(category).astype(np.int64)
    mems = np.asarray(memory_tables, dtype=np.float32)[cat[:D]]
    mf = mems.reshape(D * M, E).astype(np.float64)
    K = np.concatenate([mf @ np.asarray(W_topic, dtype=np.float64),
                        mf @ np.asarray(W_domain, dtype=np.float64)],
                       axis=0)                                   # [180, I]
    # pre-scale K by 4096 so the f32r main PSUM and the fp8 correction
    # PSUM share one scale (dKs = 4096*K - f32r(4096*K) is fp8-ranged)
    KS = K * 4096.0
    Kr = _round_f32r(KS.astype(np.float32))                      # f32r grid
    dKs = (KS - Kr.astype(np.float64)).astype(np.float32)
    dK8 = dKs.astype(ml_dtypes.float8_e4m3)

    # ktr[p, kb, n] = Kr[n, kb*128+p]
    kr_t = np.ascontiguousarray(
        Kr.T.reshape(KI, 128, KR).transpose(1, 0, 2)).reshape(128, KI * KR)
    # [p, pair, ko, m] with m padded to KRP
    dkt = dK8.T.reshape(KI // 2, 2, 128, KR).transpose(2, 0, 1, 3)
    dk_t = np.zeros((128, KI // 2, 2, KRP), ml_dtypes.float8_e4m3)
    dk_t[:, :, :, :KR] = dkt
    dk_t = np.ascontiguousarray(dk_t).reshape(128, (KI // 2) * 2 * KRP)

    norm = np.sqrt(np.einsum("bi,bi->b", feature, feature,
                             dtype=np.float64))
    r = (TAU / np.maximum(norm, 1e-12) / 4096.0).astype(np.float32)
    rsc = r.reshape(NCORES, NT, 128).transpose(0, 2, 1)          # [NC,128,NT]
    warm = np.broadcast_to(np.eye(128, dtype=np.float32)[None],
                           (NCORES, 128, 128))
    cst = np.ascontiguousarray(np.concatenate([rsc, warm], axis=2))

    # featT packed per core: [p, j, kb, b]
    fr = _round_f32r(feature)
    ft = fr.reshape(NCORES, NT, 128, KI, 128).transpose(0, 4, 1, 3, 2)
    ft = np.ascontiguousarray(ft).reshape(NCORES, 128, NT * I)
    fb8 = np.ascontiguousarray(
        ft[:, :, (NT - 1) * I:]).astype(ml_dtypes.float8_e4m3)
    return ft, cst, kr_t, dk_t, fb8


def _run(ft, cst, kr_t, dk_t, fb8, trace=False):
    from concourse.bass_utils import run_bass_kernel_spmd

    nc = _get_nc()
    in_maps = [
        {"feat": ft[c], "cst": cst[c], "ktr": kr_t, "dk8": dk_t,
         "fb8d": fb8[c]}
        for c in range(NCORES)
    ]
    res = run_bass_kernel_spmd(nc, in_maps, core_ids=list(range(NCORES)),
                               trace=trace)
    # out[p, j, d] -> [BLOC, D] per core
    outs = []
    for c in range(NCORES):
        o = res.results[c]["out"].reshape(128, NT, D)
        outs.append(o.transpose(1, 0, 2).reshape(BLOC, D))
    out = np.concatenate(outs, axis=0)
    return out.reshape(B, 1, D), res


def kernel(feature=None, W_topic=None, W_domain=None, memory_tables=None,
           category=None, **_unused):
    prep = _host_prep(feature, W_topic, W_domain, memory_tables, category)
    out, _ = _run(*prep, trace=False)
    return out
